# revision 24
# baseline (speedup 1.0000x reference)
"""BEiT-style attention (B=128, N=197, C=768, H=12) on 8 TRN2 NeuronCores.

Strategy: pure data parallelism over batch — each core processes 16
samples end-to-end; no collectives. Host pre-gathers the per-sample
bitfit biases (b_idx lookups), pre-transposes x to [C, N] per sample,
folds the attention scale into the q weights/bias, and pre-computes
exp(rel_pos_bias)^T so softmax(S + rpb) = normalize(exp(S) * exp_rpbT).

Device per sample:
  qkT  [1536,197] = w_qk @ x^T          (weights stationary, 2-sample batch)
  v    [197, 768] = x @ w_v^T + v_bias  (x^T stationary slices; bias fused
  into the single DVE psum drain)
  per head: S^T[m,n] = k_h @ q_h^T  (two heads packed in the 128-row PE
  array via tile_position), P = exp(S^T) * exp_rpbT (rpb multiply on the
  otherwise-idle GPSIMD), out^T[hd+1, n] = [v_h | 1]^T @ P  (ones column
  yields the softmax denominator), normalize with a reciprocal broadcast
  via a ones-matmul, y^T [768,197] = w_proj @ out_allT + b_proj.
Host transposes the gathered y^T back to [B, N, C].

Scheduling: one global head-iteration stream. Attention lag stages (PV
matmuls at +2, denominator broadcast + normalize at +3, projection once
a pair's 12 norms are out) and the next pair's QKV/V matmuls interleave
into every iteration and flow across pair AND repeat boundaries, so the
tensor engine sees no pipeline refill and its semaphore waits are
satisfied ahead of arrival (PE p-state stays ramped).
"""
import numpy as np
import ml_dtypes

import concourse.bass as bass
import concourse.tile as tile
from concourse import mybir
from concourse.bass_utils import run_bass_kernel_spmd

B, N, C = 128, 197, 768
H, HD = 12, 64
NCORES = 8
BL = B // NCORES          # 16 samples per core
NPAIRS = BL // 2          # 8 sample pairs
N2 = 2 * N                # 394
KT = C // 128             # 6 k-tiles
MT_QK = 2 * C // 128      # 12 m-tiles of qkT
BF16 = mybir.dt.bfloat16
F32 = mybir.dt.float32
AF = mybir.ActivationFunctionType
PV_TRAIL = 2


def _prune_dominated_waits(nc):
    """Engine queues execute in order and data sems only count up, so a
    sem-ge wait is dead if an earlier non-DMA instruction on the same
    engine already waited for >= that value. Also dedup same-sem waits
    within one instruction. DMA descriptors' waits run on the DMA queues,
    not the engine sequencer: they neither dominate nor get pruned."""
    n_drop = 0
    for f in nc.m.functions:
        for bb in f.blocks:
            tracked = {}  # engine -> {sem_id: max_ge_value_waited}
            for ins in bb.instructions:
                tname = type(ins).__name__
                is_dma = ("TensorLoad" in tname or "TensorSave" in tname
                          or "TensorCopy" in tname or "TriggeredCopy" in tname
                          or "DMACopy" in tname)
                si = ins.sync_info
                if si is None:
                    continue
                if is_dma:
                    continue
                eng = ins.engine
                tr = tracked.setdefault(eng, {})
                if any("Event" in tname or "Drain" in tname
                       or "Branch" in tname for _ in (0,)) and (
                        "Event" in tname or "Drain" in tname
                        or "Branch" in tname):
                    tracked[eng] = {}
                    continue
                waits = list(si.on_wait)
                if not waits:
                    continue
                if any(w.wait_mode != "sem-ge-imm" or w.wait_reg is not None
                       for w in waits):
                    # barrier-style waits: distrust all tracking
                    tracked = {}
                    continue
                best = {}
                for w in waits:
                    if w.id not in best or w.wait_value > best[w.id].wait_value:
                        best[w.id] = w
                kept = []
                for w in best.values():
                    if tr.get(w.id, -1) >= w.wait_value:
                        n_drop += 1
                        continue
                    kept.append(w)
                    tr[w.id] = w.wait_value
                n_drop += len(waits) - len(best)
                if len(kept) != len(waits):
                    ins.sync_info = mybir.SyncInfo(
                        on_wait=kept, on_update=list(si.on_update))
    return n_drop


def _split_sync_waits(nc, max_waits=1, max_updates=1):
    """TPB descriptors have ONE wait and ONE update slot; hoist extras
    onto same-engine NoOps (trailing-nop updates are completion-safe)."""
    n_split = 0
    for f in nc.m.functions:
        for bb in f.blocks:
            old = list(bb.instructions)
            new = []
            changed = False
            for ins in old:
                si = ins.sync_info
                tname = type(ins).__name__
                is_dma = ("TensorLoad" in tname or "TensorSave" in tname
                          or "TensorCopy" in tname or "TriggeredCopy" in tname)
                if si is None or is_dma:
                    new.append(ins)
                    continue
                waits = list(si.on_wait)
                updates = list(si.on_update)
                if len(waits) <= max_waits and len(updates) <= max_updates:
                    new.append(ins)
                    continue
                changed = True
                n_split += 1
                while len(waits) > max_waits:
                    w = waits.pop(0)
                    new.append(mybir.InstNoOp(
                        name=nc.get_next_instruction_name(), engine=ins.engine,
                        sync_info=mybir.SyncInfo(on_wait=[w], on_update=[]),
                        bass_nofuse=True))
                post = []
                while len(updates) > max_updates:
                    u = updates.pop()
                    post.append(mybir.InstNoOp(
                        name=nc.get_next_instruction_name(), engine=ins.engine,
                        sync_info=mybir.SyncInfo(on_wait=[], on_update=[u]),
                        bass_nofuse=True))
                ins.sync_info = mybir.SyncInfo(on_wait=waits, on_update=updates)
                new.append(ins)
                new.extend(post)
            if changed:
                bb.instructions = new
    return n_split


def build_nc(repeat=1):
    nc = bass.Bass("TRN2")
    xt_d = nc.declare_dram_parameter("xt", [BL, 128, KT * N], BF16, isOutput=False)
    wqkv_d = nc.declare_dram_parameter("wqkv", [128, KT, 3 * C], BF16, isOutput=False)
    wproj_d = nc.declare_dram_parameter("wproj", [128, KT, C], BF16, isOutput=False)
    rpb_d = nc.declare_dram_parameter("rpb", [N, H, N], BF16, isOutput=False)
    qkvb_d = nc.declare_dram_parameter("qkvb", [128, MT_QK, BL], F32, isOutput=False)
    projb_d = nc.declare_dram_parameter("projb", [128, KT, BL], F32, isOutput=False)
    vb_d = nc.declare_dram_parameter("vb", [BL, H * HD], BF16, isOutput=False)
    out_d = nc.declare_dram_parameter("out", [BL, C, N], F32, isOutput=True)

    with tile.TileContext(nc) as tc:
        with (
            tc.tile_pool(name="const", bufs=1) as const,
            tc.tile_pool(name="xtp", bufs=3) as xtp,
            tc.tile_pool(name="qkp", bufs=3) as qkp,
            tc.tile_pool(name="vp", bufs=4) as vp,
            tc.tile_pool(name="vbp", bufs=4) as vbp,
            tc.tile_pool(name="esp", bufs=8) as esp,
            tc.tile_pool(name="rcp", bufs=7) as rcp,
            tc.tile_pool(name="oap", bufs=3) as oap,
            tc.tile_pool(name="yp", bufs=3) as yp,
            tc.tile_pool(name="psA", bufs=2, space="PSUM") as psA,
            tc.tile_pool(name="psS", bufs=2, space="PSUM") as psS,
            tc.tile_pool(name="psB", bufs=2, space="PSUM") as psB,
        ):
            # ---- resident constants ----
            wqkv_sb = const.tile([128, KT, 3 * C], BF16)
            nc.sync.dma_start(wqkv_sb, wqkv_d[:])
            wproj_sb = const.tile([128, KT, C], BF16)
            nc.sync.dma_start(wproj_sb, wproj_d[:])
            rpb0 = const.tile([128, H, N], BF16)
            nc.sync.dma_start(rpb0, rpb_d[0:128])
            rpb1 = const.tile([69, H, N], BF16)
            nc.sync.dma_start(rpb1, rpb_d[128:N])
            ones64 = const.tile([1, 64], BF16)
            nc.vector.memset(ones64, 1.0)
            qkvb_sb = const.tile([128, MT_QK, BL], F32)
            nc.sync.dma_start(qkvb_sb, qkvb_d[:])
            projb_sb = const.tile([128, KT, BL], F32)
            nc.sync.dma_start(projb_sb, projb_d[:])

            if True:
                st = {}

                def emit_load(p):
                    sg = (2 * (p % NPAIRS), 2 * (p % NPAIRS) + 1)
                    xt = xtp.tile([128, KT, N2], BF16, name="xt")
                    for s in range(2):
                        nc.sync.dma_start(
                            xt[:, :, s * N:(s + 1) * N],
                            xt_d[sg[s]].rearrange("p (k n) -> p k n", k=KT))
                    vb_bc = [None, None]
                    for s in range(2):
                        vb_bc[s] = vbp.tile([128, H, HD], BF16, tag="vb", name="vb")
                        nc.scalar.dma_start(
                            vb_bc[s],
                            vb_d[sg[s]:sg[s] + 1, :].rearrange(
                                "o (h d) -> o h d", h=H).to_broadcast([128, H, HD]))
                    qkT = qkp.tile([128, MT_QK, N2], BF16, name="qkT")
                    st[p] = {"xt": xt, "vb": vb_bc, "qkT": qkT, "v": None}

                def emit_qkv_m(p, m):
                    sg = (2 * (p % NPAIRS), 2 * (p % NPAIRS) + 1)
                    xt, qkT = st[p]["xt"], st[p]["qkT"]
                    ps = psA.tile([128, N2], F32, tag="mm", name="ps")
                    for k in range(KT):
                        nc.tensor.matmul(ps, wqkv_sb[:, k, m * 128:(m + 1) * 128],
                                         xt[:, k, :], start=(k == 0),
                                         stop=(k == KT - 1))
                    for s in range(2):
                        dst = qkT[:, m, s * N:(s + 1) * N]
                        src = ps[:, s * N:(s + 1) * N]
                        bias = qkvb_sb[:, m, sg[s]:sg[s] + 1]
                        if m % 2 == 0:
                            nc.scalar.activation(dst, src, AF.Identity,
                                                 bias=bias, scale=1.0)
                        else:
                            nc.vector.tensor_scalar_add(dst, src, bias)

                def emit_v_chunk(p, s, nt):
                    xt, vb_bc = st[p]["xt"], st[p]["vb"]
                    if st[p]["v"] is None:
                        st[p]["v"] = [[None, None], [None, None]]
                    nts = 128 if nt == 0 else N - 128
                    vt = vp.tile([nts, H, HD + 1], BF16, tag=f"v{nt}", name="vt")
                    nc.vector.memset(vt[:, :, HD:HD + 1], 1.0)
                    for half in range(2):
                        ps = psA.tile([128, N2], F32, tag="mm", name="ps")
                        for k in range(KT):
                            nc.tensor.matmul(
                                ps[:nts, 0:384],
                                xt[:, k, s * N + nt * 128:
                                   s * N + nt * 128 + nts],
                                wqkv_sb[:, k, 2 * C + half * 384:
                                        2 * C + (half + 1) * 384],
                                start=(k == 0), stop=(k == KT - 1))
                        # fused psum drain + bitfit bias in one DVE add
                        nc.vector.tensor_add(
                            vt[:, half * 6:(half + 1) * 6, 0:HD],
                            ps[:nts, 0:384].rearrange("p (h d) -> p h d", h=6),
                            vb_bc[s][:nts, half * 6:(half + 1) * 6, :])
                    st[p]["v"][s][nt] = vt

                def emit_v(p):
                    for s in range(2):
                        for nt in range(2):
                            emit_v_chunk(p, s, nt)

                def emit_s(p, s, hp):
                    # The two concurrent row-packed matmuls drain into the
                    # two DIFFERENT banks of one 2-bank psum tile (same-bank
                    # concurrent drain is a HW error), so exp and the rpb
                    # multiply each run as ONE wide instruction.
                    qkT = st[p]["qkT"]
                    ha = 2 * hp
                    es = [None, None]
                    for mt in range(2):
                        mts = 128 if mt == 0 else N - 128
                        rpb = rpb0 if mt == 0 else rpb1
                        pss = psS.tile([128, 2, 512], F32, tag="s2", name="pss")
                        for hh, (pl, pr) in enumerate(((0, 64), (64, 128))):
                            nc.tensor.matmul(
                                pss[:mts, hh, 0:N],
                                qkT[pl:pr, KT + hp,
                                    s * N + mt * 128: s * N + mt * 128 + mts],
                                qkT[pl:pr, hp, s * N:(s + 1) * N],
                                start=True, stop=True, tile_position=(pl, 0))
                        e = esp.tile([mts, N2], BF16, tag=f"es{mt}", name="es")
                        nc.scalar.activation(
                            e.rearrange("p (a n) -> p a n", a=2),
                            pss[:mts, :, 0:N], AF.Exp)
                        # SBUF-only multiply offloaded to the idle GPSIMD
                        nc.gpsimd.tensor_mul(
                            e, e,
                            rpb[:, ha:ha + 2, :].rearrange("p a n -> p (a n)"))
                        es[mt] = e
                    return es

                def emit_pv_mm(p, s, hp, es):
                    # PV matmuls; reciprocal of the ones-column denominator
                    # goes out to DRAM so a later DMA can partition-broadcast
                    # it (PE-free, Act-free normalization).
                    v_sb = st[p]["v"]
                    pvt = psB.tile([HD + 1, N2], F32, tag="pv", name="pvt")
                    for mt in range(2):
                        for hh, h in enumerate((2 * hp, 2 * hp + 1)):
                            # start=True clears has_written for the WHOLE bank:
                            # only the very first matmul may set it.
                            nc.tensor.matmul(
                                pvt[:, hh * N:(hh + 1) * N],
                                v_sb[s][mt][:, h, :],
                                es[mt][:, hh * N:(hh + 1) * N],
                                start=(mt == 0 and hh == 0),
                                stop=(mt == 1 and hh == 1))
                    rec = rcp.tile([1, N2], BF16, tag="rc", name="rec")
                    with nc.allow_low_precision(reason="softmax denom in bf16"):
                        nc.vector.reciprocal(rec, pvt[HD:HD + 1, :])
                    return pvt, rec

                def emit_pv_bc(s, hp, pvs):
                    # broadcast 1/denom across 64 partitions via a K=1 matmul
                    # one step behind its producer so the wait is pre-satisfied
                    pvt, rec = pvs
                    bc = psA.tile([128, N2], F32, tag="mm", name="ps")
                    nc.tensor.matmul(bc[0:64, :], ones64, rec,
                                     start=True, stop=True)
                    bc_sb = rcp.tile([64, N2], BF16, tag="bc_sb", name="bc_sb")
                    nc.scalar.activation(bc_sb, bc[0:64, :], AF.Copy)
                    return bc_sb

                def emit_pv_norm(p, s, hp, pvs, bc_sb, oa):
                    pvt, _ = pvs
                    for hh in range(2):
                        nc.vector.tensor_mul(
                            oa[hh * 64:(hh + 1) * 64, hp, s * N:(s + 1) * N],
                            pvt[0:HD, hh * N:(hh + 1) * N],
                            bc_sb[:, hh * N:(hh + 1) * N])

                def emit_proj_m(p, m):
                    sg = (2 * (p % NPAIRS), 2 * (p % NPAIRS) + 1)
                    oa = st[p]["oa"]
                    ps = psA.tile([128, N2], F32, tag="mm", name="ps")
                    for k in range(KT):
                        nc.tensor.matmul(
                            ps, wproj_sb[:, k, m * 128:(m + 1) * 128],
                            oa[:, k, :], start=(k == 0), stop=(k == KT - 1))
                    y = yp.tile([128, N2], F32, tag="y", name="y")
                    for s in range(2):
                        dst = y[:, s * N:(s + 1) * N]
                        src = ps[:, s * N:(s + 1) * N]
                        bias = projb_sb[:, m, sg[s]:sg[s] + 1]
                        if m % 2 == 0:
                            nc.scalar.activation(dst, src, AF.Identity,
                                                 bias=bias, scale=1.0)
                        else:
                            nc.vector.tensor_scalar_add(dst, src, bias)
                    for s in range(2):
                        nc.sync.dma_start(
                            out_d[sg[s], m * 128:(m + 1) * 128, :],
                            y[:, s * N:(s + 1) * N])

                # Global head-iteration stream: lagged stages (PV at +2,
                # bc/norm at +3, proj when a pair's norms are all out) flow
                # across pair and repeat boundaries, so the PE never sees a
                # pipeline refill and every wait is satisfied well ahead.
                TOTAL = repeat * NPAIRS
                NH = 12
                iters = [(s, hp) for s in range(2) for hp in range(H // 2)]
                work_q = []

                def emit_work(unit):
                    kind, p_, a = unit
                    if kind == "qkv":
                        emit_qkv_m(p_, a)
                    elif kind == "v":
                        emit_v_chunk(p_, a[0], a[1])
                    elif kind == "proj":
                        emit_proj_m(p_, a)
                        if a == KT - 1:
                            del st[p_]

                emit_load(0)
                for m in range(MT_QK):
                    emit_qkv_m(0, m)
                emit_v(0)

                pend_s, pend_pv = [], []
                for g in range(NH * TOTAL):
                    p, i = divmod(g, NH)
                    if i == 0:
                        st[p]["oa"] = oap.tile([128, KT, N2], BF16, name="oa")
                        if p + 1 < TOTAL:
                            emit_load(p + 1)
                            for m in range(KT):
                                work_q.append(("qkv", p + 1, m))
                                work_q.append(("qkv", p + 1, KT + m))
                            for sv in range(2):
                                for nt in range(2):
                                    work_q.append(("v", p + 1, (sv, nt)))
                    s, hp = iters[i]
                    # oldest lag stage first: bc+norm of head g-3
                    if len(pend_pv) > 0:
                        p_, s_, hp_, pvs_ = pend_pv.pop(0)
                        bc_ = emit_pv_bc(s_, hp_, pvs_)
                        emit_pv_norm(p_, s_, hp_, pvs_, bc_, st[p_]["oa"])
                        if (s_, hp_) == iters[-1]:
                            for m in range(KT):
                                work_q.append(("proj", p_, m))
                    es = emit_s(p, s, hp)
                    for _ in range(2):
                        if work_q:
                            emit_work(work_q.pop(0))
                    if len(pend_s) > PV_TRAIL:
                        p_, s_, hp_, es_ = pend_s.pop(0)
                        pvs_ = emit_pv_mm(p_, s_, hp_, es_)
                        pend_pv.append((p_, s_, hp_, pvs_))
                    pend_s.append((p, s, hp, es))
                # drain tail
                while pend_s or pend_pv or work_q:
                    if pend_pv:
                        p_, s_, hp_, pvs_ = pend_pv.pop(0)
                        bc_ = emit_pv_bc(s_, hp_, pvs_)
                        emit_pv_norm(p_, s_, hp_, pvs_, bc_, st[p_]["oa"])
                        if (s_, hp_) == iters[-1]:
                            for m in range(KT):
                                work_q.append(("proj", p_, m))
                    for _ in range(3):
                        if work_q:
                            emit_work(work_q.pop(0))
                    if pend_s:
                        p_, s_, hp_, es_ = pend_s.pop(0)
                        pvs_ = emit_pv_mm(p_, s_, hp_, es_)
                        pend_pv.append((p_, s_, hp_, pvs_))
    n = _prune_dominated_waits(nc)
    _split_sync_waits(nc)
    return nc


_NC_CACHE = {}


def _get_nc():
    if "nc" not in _NC_CACHE:
        _NC_CACHE["nc"] = build_nc()
    return _NC_CACHE["nc"]


def _prep(x, b_idx, w_qkv, q_bias, k_bias, v_bias, rel_pos_table, rel_index,
          w_proj, b_proj):
    x = np.asarray(x, dtype=np.float32)
    b_idx = np.asarray(b_idx)
    w_qkv = np.asarray(w_qkv, dtype=np.float32)
    q_bias = np.asarray(q_bias, dtype=np.float32)
    k_bias = np.asarray(k_bias, dtype=np.float32)
    v_bias = np.asarray(v_bias, dtype=np.float32)
    rel_pos_table = np.asarray(rel_pos_table, dtype=np.float32)
    rel_index = np.asarray(rel_index)
    w_proj = np.asarray(w_proj, dtype=np.float32)
    b_proj = np.asarray(b_proj, dtype=np.float32)

    scale = HD ** (-0.5)
    # fold attention scale into q weights/bias
    w_all = w_qkv.copy()
    w_all[0:C] *= scale
    wqkvT = np.ascontiguousarray(w_all.T)                      # [C, 3C]
    wqkv_p = wqkvT.reshape(KT, 128, 3 * C).transpose(1, 0, 2)  # [128, KT, 3C]
    wprojT = np.ascontiguousarray(w_proj.T)                    # [C, C]
    wproj_p = wprojT.reshape(KT, 128, C).transpose(1, 0, 2)    # [128, KT, C]

    # per-sample gathered biases
    qk_bias = np.concatenate([q_bias * scale, k_bias], axis=1)[b_idx]  # [B, 2C]
    qkvb_all = qk_bias.T.reshape(MT_QK, 128, B)                # [12, 128, B]
    projb_all = b_proj[b_idx].T.reshape(KT, 128, B)            # [6, 128, B]
    vb_all = v_bias[b_idx]                                     # [B, C]

    # exp of transposed relative-position bias: rpbT[m, h, n] = rpb[h][n, m]
    tbl = rel_pos_table[rel_index.reshape(-1)].reshape(N, N, H)  # [n, m, h]
    rpbT = np.exp(tbl.transpose(1, 2, 0))                        # [m, h, n]
    rpb_p = np.ascontiguousarray(rpbT, dtype=np.float32).astype(ml_dtypes.bfloat16)

    # x^T packed: [B, 128, KT*N] with partition p = c % 128, free (k, n)
    xT = x.transpose(0, 2, 1)                                  # [B, C, N]
    xt_p = xT.reshape(B, KT, 128, N).transpose(0, 2, 1, 3).reshape(B, 128, KT * N)
    xt_p = xt_p.astype(ml_dtypes.bfloat16)

    wqkv_p = np.ascontiguousarray(wqkv_p).astype(ml_dtypes.bfloat16)
    wproj_p = np.ascontiguousarray(wproj_p).astype(ml_dtypes.bfloat16)

    in_maps = []
    for i in range(NCORES):
        lo, hi = i * BL, (i + 1) * BL
        in_maps.append({
            "xt": np.ascontiguousarray(xt_p[lo:hi]),
            "wqkv": wqkv_p,
            "wproj": wproj_p,
            "rpb": rpb_p,
            "qkvb": np.ascontiguousarray(qkvb_all.transpose(1, 0, 2)[:, :, lo:hi]).astype(np.float32),
            "projb": np.ascontiguousarray(projb_all.transpose(1, 0, 2)[:, :, lo:hi]).astype(np.float32),
            "vb": np.ascontiguousarray(vb_all[lo:hi]).astype(ml_dtypes.bfloat16),
        })

    return in_maps


def _gather(results):
    outT = np.concatenate([results[i]["out"] for i in range(NCORES)], axis=0)
    return np.ascontiguousarray(outT.transpose(0, 2, 1))


def kernel(**inputs):
    in_maps = _prep(**inputs)
    nc = _get_nc()
    res = run_bass_kernel_spmd(nc, in_maps, list(range(NCORES))).results
    return _gather(res)



# revision 26
# speedup vs baseline: 1.1661x; 1.1661x over previous
"""BEiT-style attention (B=128, N=197, C=768, H=12) on 8 TRN2 NeuronCores.

Strategy: pure data parallelism over batch — each core processes 16
samples end-to-end; no collectives. Host pre-gathers the per-sample
bitfit biases (b_idx lookups), pre-transposes x to [C, N] per sample,
folds the attention scale into the q weights/bias, and pre-computes
exp(rel_pos_bias)^T so softmax(S + rpb) = normalize(exp(S) * exp_rpbT).

Device per sample:
  qkT  [1536,197] = w_qk @ x^T          (weights stationary, 2-sample batch)
  v    [197, 768] = x @ w_v^T + v_bias  (x^T stationary slices; bias fused
  into the single DVE psum drain)
  per head: S^T[m,n] = k_h @ q_h^T  (two heads packed in the 128-row PE
  array via tile_position), P = exp(S^T) * exp_rpbT (rpb multiply on the
  otherwise-idle GPSIMD), out^T[hd+1, n] = [v_h | 1]^T @ P  (ones column
  yields the softmax denominator), normalize with a reciprocal broadcast
  via a ones-matmul, y^T [768,197] = w_proj @ out_allT + b_proj.
Host transposes the gathered y^T back to [B, N, C].

Scheduling: one global head-iteration stream. Attention lag stages (PV
matmuls at +2, denominator broadcast + normalize at +3, projection once
a pair's 12 norms are out) and the next pair's QKV/V matmuls interleave
into every iteration and flow across pair AND repeat boundaries, so the
tensor engine sees no pipeline refill and its semaphore waits are
satisfied ahead of arrival (PE p-state stays ramped).
"""
import numpy as np
import ml_dtypes

import concourse.bass as bass
import concourse.tile as tile
from concourse import mybir
from concourse.bass_utils import run_bass_kernel_spmd

B, N, C = 128, 197, 768
H, HD = 12, 64
NCORES = 8
BL = B // NCORES          # 16 samples per core
NPAIRS = BL // 2          # 8 sample pairs
N2 = 2 * N                # 394
KT = C // 128             # 6 k-tiles
MT_QK = 2 * C // 128      # 12 m-tiles of qkT
BF16 = mybir.dt.bfloat16
F32 = mybir.dt.float32
AF = mybir.ActivationFunctionType
PV_TRAIL = 2


def _prune_dominated_waits(nc):
    """Engine queues execute in order and data sems only count up, so a
    sem-ge wait is dead if an earlier non-DMA instruction on the same
    engine already waited for >= that value. Also dedup same-sem waits
    within one instruction. DMA descriptors' waits run on the DMA queues,
    not the engine sequencer: they neither dominate nor get pruned."""
    n_drop = 0
    for f in nc.m.functions:
        for bb in f.blocks:
            tracked = {}  # engine -> {sem_id: max_ge_value_waited}
            for ins in bb.instructions:
                tname = type(ins).__name__
                is_dma = ("TensorLoad" in tname or "TensorSave" in tname
                          or "TensorCopy" in tname or "TriggeredCopy" in tname
                          or "DMACopy" in tname)
                si = ins.sync_info
                if si is None:
                    continue
                if is_dma:
                    continue
                eng = ins.engine
                tr = tracked.setdefault(eng, {})
                if any("Event" in tname or "Drain" in tname
                       or "Branch" in tname for _ in (0,)) and (
                        "Event" in tname or "Drain" in tname
                        or "Branch" in tname):
                    tracked[eng] = {}
                    continue
                waits = list(si.on_wait)
                if not waits:
                    continue
                if any(w.wait_mode != "sem-ge-imm" or w.wait_reg is not None
                       for w in waits):
                    # barrier-style waits: distrust all tracking
                    tracked = {}
                    continue
                best = {}
                for w in waits:
                    if w.id not in best or w.wait_value > best[w.id].wait_value:
                        best[w.id] = w
                kept = []
                for w in best.values():
                    if tr.get(w.id, -1) >= w.wait_value:
                        n_drop += 1
                        continue
                    kept.append(w)
                    tr[w.id] = w.wait_value
                n_drop += len(waits) - len(best)
                if len(kept) != len(waits):
                    ins.sync_info = mybir.SyncInfo(
                        on_wait=kept, on_update=list(si.on_update))
    return n_drop


def _split_sync_waits(nc, max_waits=1, max_updates=1):
    """TPB descriptors have ONE wait and ONE update slot; hoist extras
    onto same-engine NoOps (trailing-nop updates are completion-safe)."""
    n_split = 0
    for f in nc.m.functions:
        for bb in f.blocks:
            old = list(bb.instructions)
            new = []
            changed = False
            for ins in old:
                si = ins.sync_info
                tname = type(ins).__name__
                is_dma = ("TensorLoad" in tname or "TensorSave" in tname
                          or "TensorCopy" in tname or "TriggeredCopy" in tname)
                if si is None or is_dma:
                    new.append(ins)
                    continue
                waits = list(si.on_wait)
                updates = list(si.on_update)
                if len(waits) <= max_waits and len(updates) <= max_updates:
                    new.append(ins)
                    continue
                changed = True
                n_split += 1
                while len(waits) > max_waits:
                    w = waits.pop(0)
                    new.append(mybir.InstNoOp(
                        name=nc.get_next_instruction_name(), engine=ins.engine,
                        sync_info=mybir.SyncInfo(on_wait=[w], on_update=[]),
                        bass_nofuse=True))
                post = []
                while len(updates) > max_updates:
                    u = updates.pop()
                    post.append(mybir.InstNoOp(
                        name=nc.get_next_instruction_name(), engine=ins.engine,
                        sync_info=mybir.SyncInfo(on_wait=[], on_update=[u]),
                        bass_nofuse=True))
                ins.sync_info = mybir.SyncInfo(on_wait=waits, on_update=updates)
                new.append(ins)
                new.extend(post)
            if changed:
                bb.instructions = new
    return n_split


def build_nc(repeat=1):
    nc = bass.Bass("TRN2")
    xt_d = nc.declare_dram_parameter("xt", [BL, 128, KT * N], BF16, isOutput=False)
    wqkv_d = nc.declare_dram_parameter("wqkv", [128, KT, 3 * C], BF16, isOutput=False)
    wproj_d = nc.declare_dram_parameter("wproj", [128, KT, C], BF16, isOutput=False)
    rpb_d = nc.declare_dram_parameter("rpb", [128, H // 2, 2 * N2], BF16, isOutput=False)
    qkvb_d = nc.declare_dram_parameter("qkvb", [128, MT_QK, BL], F32, isOutput=False)
    projb_d = nc.declare_dram_parameter("projb", [128, KT, BL], F32, isOutput=False)
    vb_d = nc.declare_dram_parameter("vb", [BL, H * HD], BF16, isOutput=False)
    out_d = nc.declare_dram_parameter("out", [BL, C, N], F32, isOutput=True)

    with tile.TileContext(nc) as tc:
        with (
            tc.tile_pool(name="const", bufs=1) as const,
            tc.tile_pool(name="xtp", bufs=3) as xtp,
            tc.tile_pool(name="qkp", bufs=3) as qkp,
            tc.tile_pool(name="vp", bufs=4) as vp,
            tc.tile_pool(name="vbp", bufs=4) as vbp,
            tc.tile_pool(name="esp", bufs=8) as esp,
            tc.tile_pool(name="rcp", bufs=7) as rcp,
            tc.tile_pool(name="oap", bufs=3) as oap,
            tc.tile_pool(name="yp", bufs=3) as yp,
            tc.tile_pool(name="psA", bufs=2, space="PSUM") as psA,
            tc.tile_pool(name="psS", bufs=2, space="PSUM") as psS,
            tc.tile_pool(name="psB", bufs=2, space="PSUM") as psB,
        ):
            # ---- resident constants ----
            wqkv_sb = const.tile([128, KT, 3 * C], BF16)
            nc.sync.dma_start(wqkv_sb, wqkv_d[:])
            wproj_sb = const.tile([128, KT, C], BF16)
            nc.sync.dma_start(wproj_sb, wproj_d[:])
            rpbc = const.tile([128, H // 2, 2 * N2], BF16)
            nc.sync.dma_start(rpbc, rpb_d[:])
            ones64 = const.tile([1, 64], BF16)
            nc.vector.memset(ones64, 1.0)
            qkvb_sb = const.tile([128, MT_QK, BL], F32)
            nc.sync.dma_start(qkvb_sb, qkvb_d[:])
            projb_sb = const.tile([128, KT, BL], F32)
            nc.sync.dma_start(projb_sb, projb_d[:])

            if True:
                st = {}

                def emit_load(p):
                    sg = (2 * (p % NPAIRS), 2 * (p % NPAIRS) + 1)
                    xt = xtp.tile([128, KT, N2], BF16, name="xt")
                    for s in range(2):
                        nc.sync.dma_start(
                            xt[:, :, s * N:(s + 1) * N],
                            xt_d[sg[s]].rearrange("p (k n) -> p k n", k=KT))
                    vb_bc = [None, None]
                    for s in range(2):
                        vb_bc[s] = vbp.tile([128, H, HD], BF16, tag="vb", name="vb")
                        nc.scalar.dma_start(
                            vb_bc[s],
                            vb_d[sg[s]:sg[s] + 1, :].rearrange(
                                "o (h d) -> o h d", h=H).to_broadcast([128, H, HD]))
                    qkT = qkp.tile([128, MT_QK, N2], BF16, name="qkT")
                    st[p] = {"xt": xt, "vb": vb_bc, "qkT": qkT, "v": None}

                def emit_qkv_m(p, m):
                    sg = (2 * (p % NPAIRS), 2 * (p % NPAIRS) + 1)
                    xt, qkT = st[p]["xt"], st[p]["qkT"]
                    ps = psA.tile([128, N2], F32, tag="mm", name="ps")
                    for k in range(KT):
                        nc.tensor.matmul(ps, wqkv_sb[:, k, m * 128:(m + 1) * 128],
                                         xt[:, k, :], start=(k == 0),
                                         stop=(k == KT - 1))
                    for s in range(2):
                        dst = qkT[:, m, s * N:(s + 1) * N]
                        src = ps[:, s * N:(s + 1) * N]
                        bias = qkvb_sb[:, m, sg[s]:sg[s] + 1]
                        if m % 2 == 0:
                            nc.scalar.activation(dst, src, AF.Identity,
                                                 bias=bias, scale=1.0)
                        else:
                            nc.vector.tensor_scalar_add(dst, src, bias)

                def emit_v_chunk(p, s, nt):
                    xt, vb_bc = st[p]["xt"], st[p]["vb"]
                    if st[p]["v"] is None:
                        st[p]["v"] = [[None, None], [None, None]]
                    nts = 128 if nt == 0 else N - 128
                    vt = vp.tile([nts, H, HD + 1], BF16, tag=f"v{nt}", name="vt")
                    nc.vector.memset(vt[:, :, HD:HD + 1], 1.0)
                    for half in range(2):
                        ps = psA.tile([128, N2], F32, tag="mm", name="ps")
                        for k in range(KT):
                            nc.tensor.matmul(
                                ps[:nts, 0:384],
                                xt[:, k, s * N + nt * 128:
                                   s * N + nt * 128 + nts],
                                wqkv_sb[:, k, 2 * C + half * 384:
                                        2 * C + (half + 1) * 384],
                                start=(k == 0), stop=(k == KT - 1))
                        # fused psum drain + bitfit bias in one DVE add
                        nc.vector.tensor_add(
                            vt[:, half * 6:(half + 1) * 6, 0:HD],
                            ps[:nts, 0:384].rearrange("p (h d) -> p h d", h=6),
                            vb_bc[s][:nts, half * 6:(half + 1) * 6, :])
                    st[p]["v"][s][nt] = vt

                def emit_v(p):
                    for s in range(2):
                        for nt in range(2):
                            emit_v_chunk(p, s, nt)

                def emit_s(p, s, hp):
                    # The two concurrent row-packed matmuls drain into the
                    # two DIFFERENT banks of one 2-bank psum tile. Both mt
                    # chunks land in ONE es tile so the rpb multiply is a
                    # single Pool instruction and the PV matmuls see a
                    # single producer (one PE wait instead of two).
                    qkT = st[p]["qkT"]
                    e = esp.tile([128, 2, N2], BF16, tag="es", name="es")
                    nc.gpsimd.memset(e[64:128, 1, :], 0.0)
                    for mt in range(2):
                        mts = 128 if mt == 0 else N - 128
                        pss = psS.tile([128, 2, 512], F32, tag="s2", name="pss")
                        for hh, (pl, pr) in enumerate(((0, 64), (64, 128))):
                            nc.tensor.matmul(
                                pss[:mts, hh, 0:N],
                                qkT[pl:pr, KT + hp,
                                    s * N + mt * 128: s * N + mt * 128 + mts],
                                qkT[pl:pr, hp, s * N:(s + 1) * N],
                                start=True, stop=True, tile_position=(pl, 0))
                        nc.scalar.activation(
                            e[:mts, mt, :].rearrange("p (a n) -> p a n", a=2),
                            pss[:mts, :, 0:N], AF.Exp)
                    nc.gpsimd.tensor_mul(
                        e.rearrange("p a n -> p (a n)"),
                        e.rearrange("p a n -> p (a n)"), rpbc[:, hp, :])
                    return e

                def emit_pv_mm(p, s, hp, es):
                    # PV matmuls; reciprocal of the ones-column denominator
                    # goes out to DRAM so a later DMA can partition-broadcast
                    # it (PE-free, Act-free normalization).
                    v_sb = st[p]["v"]
                    pvt = psB.tile([HD + 1, N2], F32, tag="pv", name="pvt")
                    for mt in range(2):
                        mts = 128 if mt == 0 else N - 128
                        for hh, h in enumerate((2 * hp, 2 * hp + 1)):
                            # start=True clears has_written for the WHOLE bank:
                            # only the very first matmul may set it.
                            nc.tensor.matmul(
                                pvt[:, hh * N:(hh + 1) * N],
                                v_sb[s][mt][:, h, :],
                                es[:mts, mt, hh * N:(hh + 1) * N],
                                start=(mt == 0 and hh == 0),
                                stop=(mt == 1 and hh == 1))
                    rec = rcp.tile([1, N2], BF16, tag="rc", name="rec")
                    with nc.allow_low_precision(reason="softmax denom in bf16"):
                        nc.vector.reciprocal(rec, pvt[HD:HD + 1, :])
                    return pvt, rec

                def emit_pv_bc(s, hp, pvs):
                    # broadcast 1/denom across 64 partitions via a K=1 matmul
                    # one step behind its producer so the wait is pre-satisfied
                    pvt, rec = pvs
                    bc = psA.tile([128, N2], F32, tag="mm", name="ps")
                    nc.tensor.matmul(bc[0:64, :], ones64, rec,
                                     start=True, stop=True)
                    bc_sb = rcp.tile([64, N2], BF16, tag="bc_sb", name="bc_sb")
                    nc.scalar.activation(bc_sb, bc[0:64, :], AF.Copy)
                    return bc_sb

                def emit_pv_norm(p, s, hp, pvs, bc_sb, oa):
                    pvt, _ = pvs
                    for hh in range(2):
                        nc.vector.tensor_mul(
                            oa[hh * 64:(hh + 1) * 64, hp, s * N:(s + 1) * N],
                            pvt[0:HD, hh * N:(hh + 1) * N],
                            bc_sb[:, hh * N:(hh + 1) * N])

                def emit_proj_m(p, m):
                    sg = (2 * (p % NPAIRS), 2 * (p % NPAIRS) + 1)
                    oa = st[p]["oa"]
                    ps = psA.tile([128, N2], F32, tag="mm", name="ps")
                    for k in range(KT):
                        nc.tensor.matmul(
                            ps, wproj_sb[:, k, m * 128:(m + 1) * 128],
                            oa[:, k, :], start=(k == 0), stop=(k == KT - 1))
                    y = yp.tile([128, N2], F32, tag="y", name="y")
                    for s in range(2):
                        dst = y[:, s * N:(s + 1) * N]
                        src = ps[:, s * N:(s + 1) * N]
                        bias = projb_sb[:, m, sg[s]:sg[s] + 1]
                        if m % 2 == 0:
                            nc.scalar.activation(dst, src, AF.Identity,
                                                 bias=bias, scale=1.0)
                        else:
                            nc.vector.tensor_scalar_add(dst, src, bias)
                    for s in range(2):
                        nc.sync.dma_start(
                            out_d[sg[s], m * 128:(m + 1) * 128, :],
                            y[:, s * N:(s + 1) * N])

                # Global head-iteration stream: lagged stages (PV at +2,
                # bc/norm at +3, proj when a pair's norms are all out) flow
                # across pair and repeat boundaries, so the PE never sees a
                # pipeline refill and every wait is satisfied well ahead.
                TOTAL = repeat * NPAIRS
                NH = 12
                iters = [(s, hp) for s in range(2) for hp in range(H // 2)]
                work_q = []

                def emit_work(unit):
                    kind, p_, a = unit
                    if kind == "qkv":
                        emit_qkv_m(p_, a)
                    elif kind == "v":
                        emit_v_chunk(p_, a[0], a[1])
                    elif kind == "proj":
                        emit_proj_m(p_, a)
                        if a == KT - 1:
                            del st[p_]

                emit_load(0)
                for m in range(MT_QK):
                    emit_qkv_m(0, m)
                emit_v(0)

                pend_s, pend_pv = [], []
                for g in range(NH * TOTAL):
                    p, i = divmod(g, NH)
                    if i == 0:
                        st[p]["oa"] = oap.tile([128, KT, N2], BF16, name="oa")
                        if p + 1 < TOTAL:
                            emit_load(p + 1)
                            for m in range(KT):
                                work_q.append(("qkv", p + 1, m))
                                work_q.append(("qkv", p + 1, KT + m))
                            for sv in range(2):
                                for nt in range(2):
                                    work_q.append(("v", p + 1, (sv, nt)))
                    s, hp = iters[i]
                    # oldest lag stage first: bc+norm of head g-3
                    if len(pend_pv) > 0:
                        p_, s_, hp_, pvs_ = pend_pv.pop(0)
                        bc_ = emit_pv_bc(s_, hp_, pvs_)
                        emit_pv_norm(p_, s_, hp_, pvs_, bc_, st[p_]["oa"])
                        if (s_, hp_) == iters[-1]:
                            for m in range(KT):
                                work_q.append(("proj", p_, m))
                    es = emit_s(p, s, hp)
                    for _ in range(2):
                        if work_q:
                            emit_work(work_q.pop(0))
                    if len(pend_s) > PV_TRAIL:
                        p_, s_, hp_, es_ = pend_s.pop(0)
                        pvs_ = emit_pv_mm(p_, s_, hp_, es_)
                        pend_pv.append((p_, s_, hp_, pvs_))
                    pend_s.append((p, s, hp, es))
                # drain tail
                while pend_s or pend_pv or work_q:
                    if pend_pv:
                        p_, s_, hp_, pvs_ = pend_pv.pop(0)
                        bc_ = emit_pv_bc(s_, hp_, pvs_)
                        emit_pv_norm(p_, s_, hp_, pvs_, bc_, st[p_]["oa"])
                        if (s_, hp_) == iters[-1]:
                            for m in range(KT):
                                work_q.append(("proj", p_, m))
                    for _ in range(3):
                        if work_q:
                            emit_work(work_q.pop(0))
                    if pend_s:
                        p_, s_, hp_, es_ = pend_s.pop(0)
                        pvs_ = emit_pv_mm(p_, s_, hp_, es_)
                        pend_pv.append((p_, s_, hp_, pvs_))
    n = _prune_dominated_waits(nc)
    _split_sync_waits(nc)
    return nc


_NC_CACHE = {}


def _get_nc():
    if "nc" not in _NC_CACHE:
        _NC_CACHE["nc"] = build_nc()
    return _NC_CACHE["nc"]


def _prep(x, b_idx, w_qkv, q_bias, k_bias, v_bias, rel_pos_table, rel_index,
          w_proj, b_proj):
    x = np.asarray(x, dtype=np.float32)
    b_idx = np.asarray(b_idx)
    w_qkv = np.asarray(w_qkv, dtype=np.float32)
    q_bias = np.asarray(q_bias, dtype=np.float32)
    k_bias = np.asarray(k_bias, dtype=np.float32)
    v_bias = np.asarray(v_bias, dtype=np.float32)
    rel_pos_table = np.asarray(rel_pos_table, dtype=np.float32)
    rel_index = np.asarray(rel_index)
    w_proj = np.asarray(w_proj, dtype=np.float32)
    b_proj = np.asarray(b_proj, dtype=np.float32)

    scale = HD ** (-0.5)
    # fold attention scale into q weights/bias
    w_all = w_qkv.copy()
    w_all[0:C] *= scale
    wqkvT = np.ascontiguousarray(w_all.T)                      # [C, 3C]
    wqkv_p = wqkvT.reshape(KT, 128, 3 * C).transpose(1, 0, 2)  # [128, KT, 3C]
    wprojT = np.ascontiguousarray(w_proj.T)                    # [C, C]
    wproj_p = wprojT.reshape(KT, 128, C).transpose(1, 0, 2)    # [128, KT, C]

    # per-sample gathered biases
    qk_bias = np.concatenate([q_bias * scale, k_bias], axis=1)[b_idx]  # [B, 2C]
    qkvb_all = qk_bias.T.reshape(MT_QK, 128, B)                # [12, 128, B]
    projb_all = b_proj[b_idx].T.reshape(KT, 128, B)            # [6, 128, B]
    vb_all = v_bias[b_idx]                                     # [B, C]

    # exp of transposed relative-position bias: rpbT[m, h, n] = rpb[h][n, m]
    tbl = rel_pos_table[rel_index.reshape(-1)].reshape(N, N, H)  # [n, m, h]
    rpbT = np.exp(tbl.transpose(1, 2, 0))                        # [m, h, n]
    # combined per-head-pair layout [128, H//2, (mt, hh, n)], mt=1 zero-padded
    rpbc = np.zeros((128, H // 2, 2, 2, N), dtype=np.float32)
    for hp in range(H // 2):
        for hh in range(2):
            rpbc[:, hp, 0, hh, :] = rpbT[0:128, 2 * hp + hh, :]
            rpbc[0:N - 128, hp, 1, hh, :] = rpbT[128:N, 2 * hp + hh, :]
    rpb_p = rpbc.reshape(128, H // 2, 2 * N2).astype(ml_dtypes.bfloat16)

    # x^T packed: [B, 128, KT*N] with partition p = c % 128, free (k, n)
    xT = x.transpose(0, 2, 1)                                  # [B, C, N]
    xt_p = xT.reshape(B, KT, 128, N).transpose(0, 2, 1, 3).reshape(B, 128, KT * N)
    xt_p = xt_p.astype(ml_dtypes.bfloat16)

    wqkv_p = np.ascontiguousarray(wqkv_p).astype(ml_dtypes.bfloat16)
    wproj_p = np.ascontiguousarray(wproj_p).astype(ml_dtypes.bfloat16)

    in_maps = []
    for i in range(NCORES):
        lo, hi = i * BL, (i + 1) * BL
        in_maps.append({
            "xt": np.ascontiguousarray(xt_p[lo:hi]),
            "wqkv": wqkv_p,
            "wproj": wproj_p,
            "rpb": rpb_p,
            "qkvb": np.ascontiguousarray(qkvb_all.transpose(1, 0, 2)[:, :, lo:hi]).astype(np.float32),
            "projb": np.ascontiguousarray(projb_all.transpose(1, 0, 2)[:, :, lo:hi]).astype(np.float32),
            "vb": np.ascontiguousarray(vb_all[lo:hi]).astype(ml_dtypes.bfloat16),
        })

    return in_maps


def _gather(results):
    outT = np.concatenate([results[i]["out"] for i in range(NCORES)], axis=0)
    return np.ascontiguousarray(outT.transpose(0, 2, 1))


def kernel(**inputs):
    in_maps = _prep(**inputs)
    nc = _get_nc()
    res = run_bass_kernel_spmd(nc, in_maps, list(range(NCORES))).results
    return _gather(res)

